# revision 34
# baseline (speedup 1.0000x reference)
"""Fused multi-head attention + residual + layernorm for 8 TRN2 NeuronCores.

Sharding (SPMD, no collectives in the bass kernel): core c handles batch
b = c//4 and query rows [q0, q0+512) with q0 = (c%4)*512.  Each core computes
K/V projections for its batch over the full sequence (replicated within the
4-core batch group), Q projection only for its own query rows, attention for
all 12 heads over its query rows, and the output projection.  The residual
add and layernorm run on the HOST in exact f32 (the host already holds Q):
the device ships only the pre-residual attention output, whose std is ~0.05
of the final signal, quantized to int4 with per-(core,column) scales -- so
the download is 1.6 MB and the quantization contributes only ~0.8% error.

Device layouts (SBUF partition dim first):
  qt   [768, 2048] fp8   = Q[b].T rotated so the core's own query rows come
                           first (d_model on partitions)
  q_T  [768, 512]  bf16  = per-head-stacked query projection, rows h*64+d
  k_T  [768, 2048] bf16  = key projection, rows h*64+d
  v    [128,8,2,12,80] fp8 = value projection interleaved by k-tile pair
                           for DoubleRow, + a ones column (which makes attn@v
                           also produce the softmax denominator as row 64)
  scores_T [k, q] computed per 128-row k-tile, two heads per PSUM tile,
  exp via ScalarE (scores ~ N(0,1): no max subtraction needed; bias -2 keeps
  weights inside fp8e4m3 range, softmax shift-invariance makes it exact),
  attn kept fp8, attn@v as fp8 DoubleRow matmuls (two k-tiles, contraction
  256, per matmul) accumulated in PSUM fp32, emitted two kt-slots after
  their exp so the in-order PE never blocks on ACT.

Software pipelining (emission order drives Tile's static schedule): the kt
loop of head-pair j also carries the V projection (j==0 only), the Q/K
projections of pair j+1, and the output-projection partial of pair j-1
(accumulated into an SBUF fp32 buffer so no PSUM bank is held across pairs).
LayerNorm runs at the tail, pipelined per 128-row chunk, with
rstd = rsqrt(var+eps) computed as an exp(-0.5(v-1)) seed plus Newton steps
so the whole kernel stays inside one ACT table set (no mid-kernel reload).
The tail computes per-column sums of squares (PE ones-matmul over the row
dim), turns them into int4 scales via one Sqrt activation + reciprocal,
broadcasts them back over partitions with a PE ones-matmul, and emits the
nibble-packed int4 attention output plus the bf16 scales (bitcast into the
last 4 output rows, so everything comes back in ONE fetch).

Dispatch path: the wall-clock of a warm call is dominated by the axon tunnel
(per-transfer latency ~100-200 ms, modest bandwidth), not by device compute.
So the runner here compiles the shard_map'd bass_exec jit ONCE and keeps it
(run_bass_kernel_spmd rebuilds a fresh jit each call, re-tracing and
re-lowering), keeps the replicated projection weights resident on device
(re-verified against the passed-in arrays each call, re-uploaded on change),
uploads only Q as fp8 sharded by query rows (3.15 MB), and expands it
on-device with a small jax prep jit (all_gather within each 4-core batch
group + per-core roll) that also mints the donated zero output buffers, so
no other host bytes move.  The residual + layernorm finish runs as a fused
CPU jit, with the Q + b_o part computed while the device round trip is in
flight.  Output comes back as one 1.6 MB uint8 array.
A trace path through run_bass_kernel_spmd is kept for profiling
(set kernel._CACHE["run_kwargs"] = {"trace": True, ...}).
"""

import numpy as np
import ml_dtypes
from contextlib import ExitStack

import jax
import jax.numpy as jnp
from jax.sharding import Mesh, PartitionSpec, NamedSharding

try:
    from jax import shard_map as _shard_map

    def _make_shard_map(body, mesh, in_specs, out_specs):
        return _shard_map(
            body, mesh=mesh, in_specs=in_specs, out_specs=out_specs, check_vma=False
        )
except ImportError:  # older jax
    from jax.experimental.shard_map import shard_map as _shard_map_old

    def _make_shard_map(body, mesh, in_specs, out_specs):
        return _shard_map_old(
            body, mesh=mesh, in_specs=in_specs, out_specs=out_specs, check_rep=False
        )

import concourse.bass as bass
import concourse.bacc as bacc
import concourse.tile as tile
from concourse import mybir
from concourse.bass_utils import run_bass_kernel_spmd
import concourse.bass2jax as b2j

BF16 = mybir.dt.bfloat16
F32 = mybir.dt.float32
AF = mybir.ActivationFunctionType
FP8 = mybir.dt.float8e4
VPAD = 80  # DoubleRow interleave stride must be 16B-aligned

B = 2
S = 2048
D = 768
H = 12
DH = 64
P = 128
NCORES = 8
QW = S * B // NCORES  # 512 query rows per core
CT = D // P           # 6 contraction tiles over d_model
KT = S // P           # 16 key tiles
QC = QW // P          # 4 query-row chunks of 128
NPAIR = H // 2        # heads processed in pairs (one 128-row block of k_T)
SM_SCALE = 1.0 / np.sqrt(DH)
# Schraudolph exp-to-fp8e4m3 bits: u8 = round(s*A + K), bitcast to fp8.
# A = 8*SM_SCALE/ln2; K = 8*(bias=7) - 8*2/ln2 - 0.5 (the -2 softmax shift
# and sigma=-0.5 spline-midpoint correction).  Lets DVE share the exp load.
SCHRA_A = float(8 * 0.125 / np.log(2.0))
SCHRA_K = float(56 - 16 / np.log(2.0) - 0.5)
LN_EPS = 1e-5
# int4 output quantization of the pre-residual attention output: range is
# +-C4 * rms per (core, column); q = round(clamp(x*inv_s + 7.5, 0, 15)) with
# inv_s = 7.5/(C4*rms) = A*rsqrt(colsumsq), folded into one Rsqrt activation
# via rsqrt(ssq/A^2).  Host dequantizes with s = 1/inv_s (bf16, shipped in the
# last OUT_XROWS rows of the output, bitcast to uint8).
C4 = 4.0
RSQ_SCALE = float((C4 / (7.5 * np.sqrt(QW))) ** 2)
HD = D // 2
OUT_XROWS = (D * 2) // HD  # bf16 scale bytes, in output-width rows


def build_nc() -> bass.Bass:
    nc = bacc.Bacc()
    qt8 = nc.dram_tensor("qt8", [D, S], FP8, kind="ExternalInput")
    wv8 = nc.dram_tensor("wv8", [D, D], FP8, kind="ExternalInput")
    wk8 = nc.dram_tensor("wk8", [D, D], FP8, kind="ExternalInput")
    wq8 = nc.dram_tensor("wq8", [D, D], FP8, kind="ExternalInput")
    wo8 = nc.dram_tensor("wo8", [D, D], FP8, kind="ExternalInput")
    bq = nc.dram_tensor("bq", [D], F32, kind="ExternalInput")
    bk = nc.dram_tensor("bk", [D], F32, kind="ExternalInput")
    bv = nc.dram_tensor("bv", [D], F32, kind="ExternalInput")
    # rows 0..QW-1: int4-packed attn_out (low nibble col d, high col d+384);
    # rows QW..QW+3: the per-column bf16 inv_s, bitcast to uint8
    out = nc.dram_tensor("out", [QW + OUT_XROWS, HD], mybir.dt.uint8,
                         kind="ExternalOutput")

    with tile.TileContext(nc) as tc, ExitStack() as ctx:
        singles = ctx.enter_context(tc.tile_pool(name="singles", bufs=1))
        attn_pool = ctx.enter_context(tc.tile_pool(name="attn", bufs=8))
        small_sb = ctx.enter_context(tc.tile_pool(name="small_sb", bufs=2))
        stats_pool = ctx.enter_context(tc.tile_pool(name="stats", bufs=2))
        ps_pool = ctx.enter_context(tc.tile_pool(name="ps", bufs=3, space="PSUM"))
        ps_av = ctx.enter_context(tc.tile_pool(name="ps_av", bufs=2, space="PSUM"))

        def rearr(h):
            return h[:, :].rearrange("(c p) n -> p c n", p=P)

        # --- input DMAs, ordered by first use; big tensors split so the
        # first matmuls don't wait on the whole load.  sync and gpsimd are
        # separate DMA queues and run in parallel.
        wq8_sb = singles.tile([P, CT // 2, 2, D], FP8, tag="wq8", name="wq8")
        nc.sync.dma_start(
            out=wq8_sb, in_=wq8[:, :].rearrange("(c i p) n -> p c i n", i=2, p=P)
        )
        bq_sb = singles.tile([P, CT], F32, tag="bq", name="bq")
        nc.gpsimd.dma_start(out=bq_sb, in_=bq[:].rearrange("(c p) -> p c", p=P))
        bk_sb = singles.tile([P, CT], F32, tag="bk", name="bk")
        nc.gpsimd.dma_start(out=bk_sb, in_=bk[:].rearrange("(c p) -> p c", p=P))
        bvb = singles.tile([P, D], F32, tag="bvb", name="bvb")
        nc.gpsimd.dma_start(out=bvb, in_=bv[:].partition_broadcast(P))
        wk8_sb = singles.tile([P, CT // 2, 2, D], FP8, tag="wk8", name="wk8")
        nc.sync.dma_start(
            out=wk8_sb, in_=wk8[:, :].rearrange("(c i p) n -> p c i n", i=2, p=P)
        )
        qt8_sb = singles.tile([P, CT // 2, 2, S], FP8, tag="qt8", name="qt8")
        qt8_r = qt8[:, :].rearrange("(c i p) n -> p c i n", i=2, p=P)
        nc.sync.dma_start(out=qt8_sb[:, :, :, 0:1024], in_=qt8_r[:, :, :, 0:1024])
        # fp8 ct-pair-interleaved operands for the DoubleRow V projection
        wv8_sb = singles.tile([P, CT // 2, 2, D], FP8, tag="wv8", name="wv8")
        nc.sync.dma_start(
            out=wv8_sb, in_=wv8[:, :].rearrange("(c i p) n -> p c i n", i=2, p=P)
        )
        nc.sync.dma_start(out=qt8_sb[:, :, :, 1024:S], in_=qt8_r[:, :, :, 1024:S])
        wo8_sb = singles.tile([P, CT // 2, 2, D], FP8, tag="wo8", name="wo8")
        nc.sync.dma_start(
            out=wo8_sb, in_=wo8[:, :].rearrange("(c i p) n -> p c i n", i=2, p=P)
        )

        # shift exp by e^-2 so attn weights fit fp8e4m3 (max 448); softmax is
        # shift-invariant -- the ones-column denominator scales identically
        neg2_sb = singles.tile([P, 1], F32, tag="neg2", name="neg2")
        nc.vector.memset(neg2_sb, -2.0)
        ones1 = singles.tile([1, DH], BF16, tag="ones1", name="ones1")
        nc.vector.memset(ones1, 1.0)
        # ones vectors for partition-dim reductions / broadcasts via the PE
        ones_p1 = singles.tile([P, 1], BF16, tag="ones_p1", name="ones_p1")
        nc.vector.memset(ones_p1, 1.0)
        ones_1p = singles.tile([1, P], BF16, tag="ones_1p", name="ones_1p")
        nc.vector.memset(ones_1p, 1.0)
        # rsqrt guard so an all-zero column yields a huge inv_s (saturated
        # q=15 on device, dequantized by s~0 on the host) instead of NaN
        guard = singles.tile([1, 1], F32, tag="guard", name="guard")
        nc.vector.memset(guard, 1e-20)
        # warm the ACT function table while DMAs stream
        warm_t = singles.tile([P, 1], F32, tag="warm", name="warm")
        nc.scalar.activation(warm_t, neg2_sb, AF.Exp)

        q_sb = singles.tile([P, CT, QW], BF16, tag="q_sb", name="q_sb")
        k_sb = singles.tile([P, CT, S], BF16, tag="k_sb", name="k_sb")
        v_sb = singles.tile([P, KT // 2, 2, H, VPAD], FP8, tag="v_sb", name="v_sb")
        av_sb = singles.tile([P, CT // 2, 2, QW], FP8, tag="av_sb", name="av_sb")
        # attn_out accumulator (pre-residual; the host adds Q + b_o exactly)
        x_acc = singles.tile([P, QC, D], F32, tag="x_acc", name="x_acc")
        nc.vector.memset(x_acc, 0.0)

        def q_proj(j):
            psq = ps_pool.tile([P, QW], F32, tag="ps", name="ps")
            for cp in range(CT // 2):
                nc.tensor.matmul(
                    psq,
                    wq8_sb[:, cp, :, j * P : (j + 1) * P],
                    qt8_sb[:, cp, :, 0:QW],
                    start=(cp == 0),
                    stop=(cp == CT // 2 - 1),
                    perf_mode=mybir.MatmulPerfMode.DoubleRow,
                )
            nc.vector.tensor_scalar_add(q_sb[:, j, :], psq, bq_sb[:, j : j + 1])

        def k_proj(j, n4):
            psk = ps_pool.tile([P, 512], F32, tag="ps", name="ps")
            for cp in range(CT // 2):
                nc.tensor.matmul(
                    psk,
                    wk8_sb[:, cp, :, j * P : (j + 1) * P],
                    qt8_sb[:, cp, :, n4 * 512 : (n4 + 1) * 512],
                    start=(cp == 0),
                    stop=(cp == CT // 2 - 1),
                    perf_mode=mybir.MatmulPerfMode.DoubleRow,
                )
            nc.vector.tensor_scalar_add(
                k_sb[:, j, n4 * 512 : (n4 + 1) * 512], psk, bk_sb[:, j : j + 1]
            )

        def v_proj(kt):
            psv = ps_pool.tile([P, D], F32, tag="ps", name="ps")
            for cp in range(CT // 2):
                nc.tensor.matmul(
                    psv[:, 0:512],
                    qt8_sb[:, cp, :, kt * P : (kt + 1) * P],
                    wv8_sb[:, cp, :, 0:512],
                    start=(cp == 0),
                    stop=(cp == CT // 2 - 1),
                    perf_mode=mybir.MatmulPerfMode.DoubleRow,
                )
                nc.tensor.matmul(
                    psv[:, 512:D],
                    qt8_sb[:, cp, :, kt * P : (kt + 1) * P],
                    wv8_sb[:, cp, :, 512:D],
                    start=(cp == 0),
                    stop=(cp == CT // 2 - 1),
                    perf_mode=mybir.MatmulPerfMode.DoubleRow,
                )
            nc.vector.memset(v_sb[:, kt // 2, kt % 2, :, DH : DH + 1], 1.0)
            with nc.allow_low_precision(
                reason="fp8 attn@v operands; error diluted by layernorm"
            ):
                nc.vector.tensor_add(
                    v_sb[:, kt // 2, kt % 2, :, 0:DH],
                    psv.rearrange("p (h d) -> p h d", h=H),
                    bvb.rearrange("p (h d) -> p h d", h=H),
                )

        def o_proj(jp, qc):
            # pair-group jp's (two head pairs) contribution to output rows
            # [qc*128, (qc+1)*128), DoubleRow over the pair interleave,
            # accumulated into x_acc (fp32 SBUF) so PSUM is freed per chunk
            pso = ps_pool.tile([P, D], F32, tag="ps", name="ps")
            nc.tensor.matmul(
                pso[:, 0:512],
                av_sb[:, jp, :, qc * P : (qc + 1) * P],
                wo8_sb[:, jp, :, 0:512],
                start=True,
                stop=True,
                perf_mode=mybir.MatmulPerfMode.DoubleRow,
            )
            nc.tensor.matmul(
                pso[:, 512:D],
                av_sb[:, jp, :, qc * P : (qc + 1) * P],
                wo8_sb[:, jp, :, 512:D],
                start=True,
                stop=True,
                perf_mode=mybir.MatmulPerfMode.DoubleRow,
            )
            nc.vector.tensor_add(x_acc[:, qc, :], x_acc[:, qc, :], pso)

        # initial projections for pair 0 (rest is pipelined into the loop)
        q_proj(0)
        k_proj(0, 0)
        v_proj(0)
        v_proj(1)

        def emit_av(j, ktp, avs, at_tiles):
            # attn@v for k-tile pair ktp, emitted 2 kts after its exps so the
            # in-order PE never blocks waiting on ACT output
            for r in range(2):
                nc.tensor.matmul(
                    avs[r],
                    v_sb[:, ktp, :, 2 * j + r, 0 : DH + 1],
                    at_tiles[ktp][:, :, r * QW : (r + 1) * QW],
                    start=(ktp == 0),
                    stop=(ktp == KT // 2 - 1),
                    perf_mode=mybir.MatmulPerfMode.DoubleRow,
                )

        def emit_norm(j, avs, chunked):
            # normalize: row DH of av is the softmax denominator per q column
            rcs, rbss = [], []
            for r in range(2):
                rc = small_sb.tile([1, QW], BF16, tag="recip", name="recip")
                with nc.allow_low_precision(
                    reason="bf16 softmax denominators; error diluted by layernorm"
                ):
                    nc.vector.reciprocal(rc, avs[r][DH : DH + 1, :])
                rcs.append(rc)
            for r in range(2):
                rbp = ps_pool.tile([DH, QW], F32, tag="ps", name="ps")
                nc.tensor.matmul(rbp, ones1, rcs[r], start=True, stop=True)
                rbs = small_sb.tile([DH, QW], F32, tag="rb", name="rb")
                nc.vector.tensor_copy(rbs, rbp)
                rbss.append(rbs)
            with nc.allow_low_precision(
                reason="fp8 attn output for DoubleRow output projection"
            ):
                if not chunked:
                    for r in range(2):
                        nc.vector.tensor_mul(
                            av_sb[r * DH : (r + 1) * DH, j // 2, j % 2, :],
                            avs[r][0:DH, :],
                            rbss[r],
                        )
                else:
                    for qc in range(QC):
                        for r in range(2):
                            nc.vector.tensor_mul(
                                av_sb[r * DH : (r + 1) * DH, j // 2, j % 2, qc * P : (qc + 1) * P],
                                avs[r][0:DH, qc * P : (qc + 1) * P],
                                rbss[r][:, qc * P : (qc + 1) * P],
                            )

        prev = None  # (j, avs) of the previous pair, normalized inside this one
        for j in range(NPAIR):
            av0 = ps_av.tile([DH + 1, QW], F32, tag="av", name="av")
            av1 = ps_av.tile([DH + 1, QW], F32, tag="av", name="av")
            avs = (av0, av1)
            at_tiles = {}

            for kt in range(KT):
                if j == 0 and kt < KT - 2:
                    v_proj(kt + 2)
                if j == 0 and kt in (1, 3, 5):
                    k_proj(0, (kt + 1) // 2)
                pss = ps_pool.tile([P, 2 * QW], F32, tag="ps", name="ps")
                for r in range(2):
                    nc.tensor.matmul(
                        pss[:, r * QW : (r + 1) * QW],
                        k_sb[r * DH : (r + 1) * DH, j, kt * P : (kt + 1) * P],
                        q_sb[r * DH : (r + 1) * DH, j, :],
                        start=True,
                        stop=True,
                    )
                if kt % 2 == 0:
                    at_tiles[kt // 2] = attn_pool.tile(
                        [P, 2, 2 * QW], FP8, tag="at", name="at"
                    )
                if 1 <= j <= 5 and kt in (3, 6, 10):
                    # offload this tile's exp to DVE via the Schraudolph
                    # bit-trick (uint8 convert saturates negatives to zero)
                    with nc.allow_low_precision(
                        reason="Schraudolph fp8 attn weights; diluted by layernorm"
                    ):
                        nc.vector.tensor_scalar(
                            out=at_tiles[kt // 2][:, kt % 2, :].bitcast(
                                mybir.dt.uint8
                            ),
                            in0=pss,
                            scalar1=SCHRA_A,
                            scalar2=SCHRA_K,
                            op0=mybir.AluOpType.mult,
                            op1=mybir.AluOpType.add,
                        )
                else:
                    nc.scalar.activation(
                        at_tiles[kt // 2][:, kt % 2, :], pss, AF.Exp,
                        scale=SM_SCALE, bias=neg2_sb,
                    )
                if kt == 1 and prev is not None:
                    emit_norm(prev[0], prev[1], chunked=False)
                    prev = None
                if kt % 2 == 1 and kt >= 3:
                    emit_av(j, kt // 2 - 1, avs, at_tiles)
                if j < NPAIR - 1:
                    if kt == 7:
                        q_proj(j + 1)
                    elif kt in (9, 11, 13, 15):
                        k_proj(j + 1, (kt - 9) // 2)
                if j >= 2 and j % 2 == 0 and kt in (4, 7, 12, 14):
                    o_proj(j // 2 - 1, (4, 7, 12, 14).index(kt))

            emit_av(j, KT // 2 - 1, avs, at_tiles)
            prev = (j, avs)

        # last pair: reciprocal + broadcast once, then per-chunk
        # normalize -> output projection -> layernorm, fully pipelined
        lavs = prev[1]
        lrbss = []
        for r in range(2):
            rc = small_sb.tile([1, QW], BF16, tag="recip", name="recip")
            with nc.allow_low_precision(
                reason="bf16 softmax denominators; error diluted by layernorm"
            ):
                nc.vector.reciprocal(rc, lavs[r][DH : DH + 1, :])
            rbp = ps_pool.tile([DH, QW], F32, tag="ps", name="ps")
            nc.tensor.matmul(rbp, ones1, rc, start=True, stop=True)
            rbs = small_sb.tile([DH, QW], F32, tag="rb", name="rb")
            nc.vector.tensor_copy(rbs, rbp)
            lrbss.append(rbs)

        # pass 1 over the chunks: finish attn_out = x_acc + last o_proj and
        # accumulate per-column sums of squares (PE ones-matmul reduces over
        # the partition/row dim; accumulation across chunks lives in SBUF so
        # no PSUM bank is pinned across the loop)
        cs_acc = stats_pool.tile([1, D], F32, tag="cs_acc", name="cs_acc")
        for qc in range(QC):
            with nc.allow_low_precision(
                reason="fp8 attn output for DoubleRow output projection"
            ):
                for r in range(2):
                    nc.vector.tensor_mul(
                        av_sb[r * DH : (r + 1) * DH, NPAIR // 2 - 1, 1, qc * P : (qc + 1) * P],
                        lavs[r][0:DH, qc * P : (qc + 1) * P],
                        lrbss[r][:, qc * P : (qc + 1) * P],
                    )
            pso = ps_pool.tile([P, D], F32, tag="ps", name="ps")
            nc.tensor.matmul(
                pso[:, 0:512],
                av_sb[:, NPAIR // 2 - 1, :, qc * P : (qc + 1) * P],
                wo8_sb[:, NPAIR // 2 - 1, :, 0:512],
                start=True,
                stop=True,
                perf_mode=mybir.MatmulPerfMode.DoubleRow,
            )
            nc.tensor.matmul(
                pso[:, 512:D],
                av_sb[:, NPAIR // 2 - 1, :, qc * P : (qc + 1) * P],
                wo8_sb[:, NPAIR // 2 - 1, :, 512:D],
                start=True,
                stop=True,
                perf_mode=mybir.MatmulPerfMode.DoubleRow,
            )
            x = x_acc[:, qc, :]
            nc.vector.tensor_add(x, x, pso)
            sq = stats_pool.tile([P, D], BF16, tag="sq_scr", name="sq_scr", bufs=2)
            with nc.allow_low_precision(
                reason="bf16 squares only set the int4 quantization scale"
            ):
                nc.scalar.activation(sq, x, AF.Square)
            ps_cs = ps_pool.tile([1, D], F32, tag="ps", name="ps")
            # split at the PSUM bank boundary (512 f32 per bank per matmul)
            nc.tensor.matmul(ps_cs[:, 0:512], ones_p1, sq[:, 0:512], start=True, stop=True)
            nc.tensor.matmul(ps_cs[:, 512:D], ones_p1, sq[:, 512:D], start=True, stop=True)
            if qc == 0:
                nc.vector.tensor_copy(cs_acc, ps_cs)
            else:
                nc.vector.tensor_add(cs_acc, cs_acc, ps_cs)

        # inv_s = (7.5*sqrt(QW)/C4) * rsqrt(colsumsq): sqrt(ssq/A^2) then a
        # reciprocal (bass blocks the Rsqrt ACT function for accuracy); bf16
        # so the host can reproduce the exact divisor from the shipped bits
        srt = stats_pool.tile([1, D], F32, tag="srt", name="srt")
        nc.scalar.activation(srt, cs_acc, AF.Sqrt, scale=RSQ_SCALE, bias=guard)
        inv_s = stats_pool.tile([1, D], BF16, tag="inv_s", name="inv_s")
        with nc.allow_low_precision(
            reason="bf16 quantization scale; host dequantizes with same bits"
        ):
            nc.vector.reciprocal(inv_s, srt)
        ps_b = ps_pool.tile([P, D], F32, tag="ps", name="ps")
        nc.tensor.matmul(ps_b[:, 0:512], ones_1p, inv_s[:, 0:512], start=True, stop=True)
        nc.tensor.matmul(ps_b[:, 512:D], ones_1p, inv_s[:, 512:D], start=True, stop=True)

        # pass 2: quantize to int4 (offset-binary, saturating convert handles
        # clamp-at-0; explicit min handles clamp-at-15), pack column d with
        # column d+384 into one byte, ship
        for qc in range(QC):
            x = x_acc[:, qc, :]
            tt = stats_pool.tile([P, D], F32, tag="tt_scr", name="tt_scr", bufs=2)
            nc.vector.tensor_mul(tt, x, ps_b)
            qu = stats_pool.tile([P, D], mybir.dt.uint8, tag="qu_scr", name="qu_scr", bufs=2)
            with nc.allow_low_precision(
                reason="int4 output quantization, ~0.8% of the 2e-2 gate"
            ):
                nc.vector.tensor_scalar(
                    out=qu, in0=tt, scalar1=7.5, scalar2=15.0,
                    op0=mybir.AluOpType.add, op1=mybir.AluOpType.min,
                )
            qf = stats_pool.tile([P, D], F32, tag="qf_scr", name="qf_scr", bufs=2)
            nc.vector.tensor_copy(qf, qu)  # rounded levels, exact in f32
            pk = stats_pool.tile([P, HD], mybir.dt.uint8, tag="pk_scr", name="pk_scr", bufs=2)
            with nc.allow_low_precision(
                reason="int4 nibble packing; values are exact small integers"
            ):
                nc.vector.scalar_tensor_tensor(
                    out=pk, in0=qf[:, HD:D], scalar=16.0, in1=qf[:, 0:HD],
                    op0=mybir.AluOpType.mult, op1=mybir.AluOpType.add,
                )
            nc.sync.dma_start(out=out[qc * P : (qc + 1) * P, :], in_=pk)
        # ship the bf16 scales as the tail rows, bitcast to uint8 (one DMA
        # per row: the SBUF source lives on a single partition)
        inv_u8 = inv_s.bitcast(mybir.dt.uint8)
        for r in range(OUT_XROWS):
            nc.sync.dma_start(
                out=out[QW + r : QW + r + 1, :],
                in_=inv_u8[:, r * HD : (r + 1) * HD],
            )

    nc.finalize()
    return nc


_CACHE: dict = {}
_BF = ml_dtypes.bfloat16
_FP8 = ml_dtypes.float8_e4m3


def _setup():
    """Build the bass module, the persistent kernel jit and the prep jit."""
    nc = build_nc()
    b2j.install_neuronx_cc_hook()

    partition_name = nc.partition_id_tensor.name if nc.partition_id_tensor else None
    in_names, out_names, out_avals = [], [], []
    for alloc in nc.m.functions[0].allocations:
        if not isinstance(alloc, mybir.MemoryLocationSet):
            continue
        name = alloc.memorylocations[0].name
        if alloc.kind == "ExternalInput":
            if name != partition_name:
                in_names.append(name)
        elif alloc.kind == "ExternalOutput":
            out_names.append(name)
            out_avals.append(
                jax.core.ShapedArray(tuple(alloc.tensor_shape), mybir.dt.np(alloc.dtype))
            )
    n_params = len(in_names)
    n_outs = len(out_names)
    in_names_all = in_names + out_names + ([partition_name] if partition_name else [])
    donate = tuple(range(n_params, n_params + n_outs))

    def _body(*args):
        operands = list(args)
        if partition_name is not None:
            operands.append(b2j.partition_id_tensor())
        outs = b2j._bass_exec_p.bind(
            *operands,
            out_avals=tuple(out_avals),
            in_names=tuple(in_names_all),
            out_names=tuple(out_names),
            lowering_input_output_aliases=(),
            sim_require_finite=True,
            sim_require_nnan=True,
            nc=nc,
        )
        return tuple(outs)

    devices = jax.devices()[:NCORES]
    mesh = Mesh(np.asarray(devices), ("core",))
    pcore = PartitionSpec("core")
    sharding = NamedSharding(mesh, pcore)
    jitted = jax.jit(
        _make_shard_map(
            _body,
            mesh=mesh,
            in_specs=(pcore,) * (n_params + n_outs),
            out_specs=(pcore,) * n_outs,
        ),
        donate_argnums=donate,
        keep_unused=True,
    )

    def _prep(qlocal):
        # per-core [QW, D] fp8 (the core's own query rows) -> rotated fp8
        # Q^T over the full batch sequence + the donated zero output buffer
        g = jax.lax.all_gather(
            qlocal,
            "core",
            axis_index_groups=[[0, 1, 2, 3], [4, 5, 6, 7]],
            tiled=True,
        )  # [S, D] = the core's whole batch, in row order
        q0 = (jax.lax.axis_index("core") % 4) * QW
        g2 = jnp.concatenate([g, g], axis=0)
        rolled = jax.lax.dynamic_slice(g2, (q0, 0), (S, D))
        qt8 = rolled.T
        zeros = jnp.zeros((QW + OUT_XROWS, HD), jnp.uint8)
        return qt8, zeros

    prep = jax.jit(
        _make_shard_map(
            _prep, mesh=mesh, in_specs=(pcore,), out_specs=(pcore,) * 2
        )
    )

    cpu = jax.local_devices(backend="cpu")[0]

    def _pre(q2d, bo):
        return q2d + bo

    def _post(fetched, qbo, gamma, beta):
        d = fetched.reshape(NCORES, QW + OUT_XROWS, HD)
        packed = d[:, :QW, :]
        inv_s = jax.lax.bitcast_convert_type(
            d[:, QW:, :].reshape(NCORES, D, 2), jnp.bfloat16
        )  # [NCORES, D]
        s = 1.0 / inv_s.astype(jnp.float32)
        lo = (packed & 15).astype(jnp.float32) - 7.5
        hi = (packed >> 4).astype(jnp.float32) - 7.5
        deq = jnp.concatenate([lo, hi], axis=2) * s[:, None, :]
        x = qbo.reshape(NCORES, QW, D) + deq
        x = x.reshape(NCORES * QW, D)
        mu = x.mean(-1, keepdims=True)
        xc = x - mu
        var = (xc * xc).mean(-1, keepdims=True)
        y = xc * jax.lax.rsqrt(var + LN_EPS) * gamma + beta
        return y.reshape(B, S, D)

    with jax.default_device(cpu):
        pre = jax.jit(_pre)
        post = jax.jit(_post)

    _CACHE.update(
        nc=nc,
        jitted=jitted,
        prep=prep,
        pre=pre,
        post=post,
        cpu=cpu,
        sharding=sharding,
        in_names=in_names,
    )


def _static_inputs(inputs) -> dict:
    """Device-resident replicated weights/biases; re-uploaded if they change."""
    names = ("W_q", "W_k", "W_v", "W_o", "b_q", "b_k", "b_v")
    ids = _CACHE.get("static_ids")
    if ids is not None and all(inputs[k] is ids[k] for k in names):
        return _CACHE["static_dev"]
    host = {k: np.asarray(inputs[k], dtype=np.float32) for k in names}
    cached = _CACHE.get("static_host")
    if cached is not None and all(np.array_equal(host[k], cached[k]) for k in names):
        _CACHE["static_ids"] = {k: inputs[k] for k in names}
        return _CACHE["static_dev"]

    sh = _CACHE["sharding"]
    tiled8 = lambda a: np.broadcast_to(a, (NCORES,) + a.shape).reshape(
        NCORES * a.shape[0], *a.shape[1:]
    )
    wT8 = lambda k: tiled8(
        np.ascontiguousarray(host[k].T).astype(_BF).astype(_FP8)
    )
    dev_host = {
        "wq8": wT8("W_q"),
        "wk8": wT8("W_k"),
        "wv8": wT8("W_v"),
        "wo8": wT8("W_o"),
        "bq": tiled8(host["b_q"]),
        "bk": tiled8(host["b_k"]),
        "bv": tiled8(host["b_v"]),
    }
    keys = list(dev_host)
    devs = jax.device_put([dev_host[k] for k in keys], [sh] * len(keys))
    static_dev = dict(zip(keys, devs))
    _CACHE["static_host"] = host
    _CACHE["static_ids"] = {k: inputs[k] for k in names}
    _CACHE["static_dev"] = static_dev
    return static_dev


def _kernel_traced(inputs) -> np.ndarray:
    """Profiling path through run_bass_kernel_spmd (host-side prep)."""
    Q = np.asarray(inputs["Q"], dtype=np.float32)
    f32 = lambda k: np.ascontiguousarray(np.asarray(inputs[k], dtype=np.float32))
    wT8 = lambda k: np.ascontiguousarray(
        np.asarray(inputs[k], np.float32).T
    ).astype(_BF).astype(_FP8)
    Wq8, Wk8, Wv8, Wo8 = wT8("W_q"), wT8("W_k"), wT8("W_v"), wT8("W_o")
    QT = [np.ascontiguousarray(Q[b].T).astype(_BF).astype(_FP8) for b in range(B)]
    in_maps = []
    for c in range(NCORES):
        b, q0 = c // 4, (c % 4) * QW
        qt_rot = np.ascontiguousarray(
            np.concatenate([QT[b][:, q0:], QT[b][:, :q0]], axis=1)
        )
        in_maps.append(
            {
                "qt8": qt_rot,
                "wq8": Wq8, "wk8": Wk8, "wv8": Wv8, "wo8": Wo8,
                "bq": f32("b_q"), "bk": f32("b_k"), "bv": f32("b_v"),
            }
        )
    res = run_bass_kernel_spmd(
        _CACHE["nc"], in_maps, core_ids=list(range(NCORES)),
        **_CACHE.get("run_kwargs", {}),
    )
    _CACHE["last_result"] = res
    fetched = np.concatenate(
        [np.asarray(res.results[c]["out"]) for c in range(NCORES)], axis=0
    )
    q2d = Q.reshape(NCORES * QW, D)
    with jax.default_device(_CACHE["cpu"]):
        qbo = _CACHE["pre"](q2d, f32("b_o"))
        out = _CACHE["post"](fetched, qbo, f32("ln_gamma"), f32("ln_beta"))
    return np.asarray(out)


def kernel(**inputs) -> np.ndarray:
    if "nc" not in _CACHE:
        _setup()
    if _CACHE.get("run_kwargs"):
        return _kernel_traced(inputs)

    sh = _CACHE["sharding"]

    # issue the Q upload and device-side prep first -- they are the head of
    # the critical path -- then do all remaining host work (static-input
    # resolution, the exact f32 Q + b_o residual) while the device runs.
    # core c <-> global row block c*QW: row order matches Q's (batch-major)
    q2d = np.asarray(inputs["Q"], dtype=np.float32).reshape(NCORES * QW, D)
    q8 = q2d.astype(_BF).astype(_FP8)
    qdev = jax.device_put(q8, sh)
    qt8_d, zeros_d = _CACHE["prep"](qdev)

    static_dev = _static_inputs(inputs)
    feed = dict(static_dev)
    feed["qt8"] = qt8_d
    args = [feed[name] for name in _CACHE["in_names"]]
    (out_d,) = _CACHE["jitted"](*args, zeros_d)
    out_d.copy_to_host_async()  # enqueue D2H before blocking on it

    # overlapped with the device round trip: the exact residual on CPU
    with jax.default_device(_CACHE["cpu"]):
        qbo = _CACHE["pre"](q2d, np.asarray(inputs["b_o"], dtype=np.float32))

    fetched = np.asarray(out_d)  # [NCORES*(QW+4), 384] uint8, blocks
    with jax.default_device(_CACHE["cpu"]):
        out = _CACHE["post"](
            fetched,
            qbo,
            np.asarray(inputs["ln_gamma"], dtype=np.float32),
            np.asarray(inputs["ln_beta"], dtype=np.float32),
        )
    return np.asarray(out)


# revision 37
# speedup vs baseline: 1.1695x; 1.1695x over previous
"""Fused multi-head attention + residual + layernorm for 8 TRN2 NeuronCores.

Sharding (SPMD, no collectives in the bass kernel): core c handles batch
b = c//4 and query rows [q0, q0+512) with q0 = (c%4)*512.  Each core computes
K/V projections for its batch over the full sequence (replicated within the
4-core batch group), Q projection only for its own query rows, attention for
all 12 heads over its query rows, and the output projection.  The residual
add and layernorm run on the HOST in exact f32 (the host already holds Q):
the device ships only the pre-residual attention output, whose std is ~0.05
of the final signal, quantized to int4 with per-(core,column) scales -- so
the download is 1.6 MB and the quantization contributes only ~0.8% error.

Device layouts (SBUF partition dim first):
  qt   [768, 2048] fp8   = Q[b].T rotated so the core's own query rows come
                           first (d_model on partitions)
  q_T  [768, 512]  bf16  = per-head-stacked query projection, rows h*64+d
  k_T  [768, 2048] bf16  = key projection, rows h*64+d
  v    [128,8,2,12,80] fp8 = value projection interleaved by k-tile pair
                           for DoubleRow, + a ones column (which makes attn@v
                           also produce the softmax denominator as row 64)
  scores_T [k, q] computed per 128-row k-tile, two heads per PSUM tile,
  exp via ScalarE (scores ~ N(0,1): no max subtraction needed; bias -2 keeps
  weights inside fp8e4m3 range, softmax shift-invariance makes it exact),
  attn kept fp8, attn@v as fp8 DoubleRow matmuls (two k-tiles, contraction
  256, per matmul) accumulated in PSUM fp32, emitted two kt-slots after
  their exp so the in-order PE never blocks on ACT.

Software pipelining (emission order drives Tile's static schedule): the kt
loop of head-pair j also carries the V projection (j==0 only), the Q/K
projections of pair j+1, and the output-projection partial of pair j-1
(accumulated into an SBUF fp32 buffer so no PSUM bank is held across pairs).
LayerNorm runs at the tail, pipelined per 128-row chunk, with
rstd = rsqrt(var+eps) computed as an exp(-0.5(v-1)) seed plus Newton steps
so the whole kernel stays inside one ACT table set (no mid-kernel reload).
The tail computes per-column sums of squares (PE ones-matmul over the row
dim), turns them into int4 scales via one Sqrt activation + reciprocal,
broadcasts them back over partitions with a PE ones-matmul, and emits the
nibble-packed int4 attention output plus the bf16 scales (bitcast into the
last 4 output rows, so everything comes back in ONE fetch).

Dispatch path: the wall-clock of a warm call is dominated by the axon tunnel
(per-transfer latency ~100-200 ms, modest bandwidth), not by device compute.
So the runner here compiles the shard_map'd bass_exec jit ONCE and keeps it
(run_bass_kernel_spmd rebuilds a fresh jit each call, re-tracing and
re-lowering), keeps the replicated projection weights resident on device
(re-verified against the passed-in arrays each call, re-uploaded on change),
uploads only Q as fp8 sharded by query rows (3.15 MB), and expands it
on-device with a small jax prep jit (all_gather within each 4-core batch
group + per-core roll) that also mints the donated zero output buffers, so
no other host bytes move.  The residual + layernorm finish runs as a fused
CPU jit, with the Q + b_o part computed while the device round trip is in
flight.  Output comes back as one 1.6 MB uint8 array.
A trace path through run_bass_kernel_spmd is kept for profiling
(set kernel._CACHE["run_kwargs"] = {"trace": True, ...}).
"""

import numpy as np
import ml_dtypes
from contextlib import ExitStack

import jax
import jax.numpy as jnp
from jax.sharding import Mesh, PartitionSpec, NamedSharding

try:
    from jax import shard_map as _shard_map

    def _make_shard_map(body, mesh, in_specs, out_specs):
        return _shard_map(
            body, mesh=mesh, in_specs=in_specs, out_specs=out_specs, check_vma=False
        )
except ImportError:  # older jax
    from jax.experimental.shard_map import shard_map as _shard_map_old

    def _make_shard_map(body, mesh, in_specs, out_specs):
        return _shard_map_old(
            body, mesh=mesh, in_specs=in_specs, out_specs=out_specs, check_rep=False
        )

import concourse.bass as bass
import concourse.bacc as bacc
import concourse.tile as tile
from concourse import mybir
from concourse.bass_utils import run_bass_kernel_spmd
import concourse.bass2jax as b2j

BF16 = mybir.dt.bfloat16
F32 = mybir.dt.float32
AF = mybir.ActivationFunctionType
FP8 = mybir.dt.float8e4
VPAD = 80  # DoubleRow interleave stride must be 16B-aligned

B = 2
S = 2048
D = 768
H = 12
DH = 64
P = 128
NCORES = 8
QW = S * B // NCORES  # 512 query rows per core
CT = D // P           # 6 contraction tiles over d_model
KT = S // P           # 16 key tiles
QC = QW // P          # 4 query-row chunks of 128
NPAIR = H // 2        # heads processed in pairs (one 128-row block of k_T)
SM_SCALE = 1.0 / np.sqrt(DH)
# Schraudolph exp-to-fp8e4m3 bits: u8 = round(s*A + K), bitcast to fp8.
# A = 8*SM_SCALE/ln2; K = 8*(bias=7) - 8*2/ln2 - 0.5 (the -2 softmax shift
# and sigma=-0.5 spline-midpoint correction).  Lets DVE share the exp load.
SCHRA_A = float(8 * 0.125 / np.log(2.0))
SCHRA_K = float(56 - 16 / np.log(2.0) - 0.5)
LN_EPS = 1e-5
# int4 output quantization of the pre-residual attention output: range is
# +-C4 * rms per (core, column); q = round(clamp(x*inv_s + 7.5, 0, 15)) with
# inv_s = 7.5/(C4*rms) = A*rsqrt(colsumsq), folded into one Rsqrt activation
# via rsqrt(ssq/A^2).  Host dequantizes with s = 1/inv_s (bf16, shipped in the
# last OUT_XROWS rows of the output, bitcast to uint8).
C4 = 4.0
RSQ_SCALE = float((C4 / (7.5 * np.sqrt(QW))) ** 2)
HD = D // 2
OUT_XROWS = (D * 2) // HD  # bf16 scale bytes, in output-width rows


def build_nc() -> bass.Bass:
    nc = bacc.Bacc()
    qt8 = nc.dram_tensor("qt8", [D, S], FP8, kind="ExternalInput")
    wv8 = nc.dram_tensor("wv8", [D, D], FP8, kind="ExternalInput")
    wk8 = nc.dram_tensor("wk8", [D, D], FP8, kind="ExternalInput")
    wq8 = nc.dram_tensor("wq8", [D, D], FP8, kind="ExternalInput")
    wo8 = nc.dram_tensor("wo8", [D, D], FP8, kind="ExternalInput")
    bq = nc.dram_tensor("bq", [D], F32, kind="ExternalInput")
    bk = nc.dram_tensor("bk", [D], F32, kind="ExternalInput")
    bv = nc.dram_tensor("bv", [D], F32, kind="ExternalInput")
    # rows 0..QW-1: int4-packed attn_out (low nibble col d, high col d+384);
    # rows QW..QW+3: the per-column bf16 inv_s, bitcast to uint8
    out = nc.dram_tensor("out", [QW + OUT_XROWS, HD], mybir.dt.uint8,
                         kind="ExternalOutput")

    with tile.TileContext(nc) as tc, ExitStack() as ctx:
        singles = ctx.enter_context(tc.tile_pool(name="singles", bufs=1))
        attn_pool = ctx.enter_context(tc.tile_pool(name="attn", bufs=8))
        small_sb = ctx.enter_context(tc.tile_pool(name="small_sb", bufs=2))
        stats_pool = ctx.enter_context(tc.tile_pool(name="stats", bufs=2))
        ps_pool = ctx.enter_context(tc.tile_pool(name="ps", bufs=3, space="PSUM"))
        ps_av = ctx.enter_context(tc.tile_pool(name="ps_av", bufs=2, space="PSUM"))

        def rearr(h):
            return h[:, :].rearrange("(c p) n -> p c n", p=P)

        # --- input DMAs, ordered by first use; big tensors split so the
        # first matmuls don't wait on the whole load.  sync and gpsimd are
        # separate DMA queues and run in parallel.
        wq8_sb = singles.tile([P, CT // 2, 2, D], FP8, tag="wq8", name="wq8")
        nc.sync.dma_start(
            out=wq8_sb, in_=wq8[:, :].rearrange("(c i p) n -> p c i n", i=2, p=P)
        )
        bq_sb = singles.tile([P, CT], F32, tag="bq", name="bq")
        nc.gpsimd.dma_start(out=bq_sb, in_=bq[:].rearrange("(c p) -> p c", p=P))
        bk_sb = singles.tile([P, CT], F32, tag="bk", name="bk")
        nc.gpsimd.dma_start(out=bk_sb, in_=bk[:].rearrange("(c p) -> p c", p=P))
        bvb = singles.tile([P, D], F32, tag="bvb", name="bvb")
        nc.gpsimd.dma_start(out=bvb, in_=bv[:].partition_broadcast(P))
        wk8_sb = singles.tile([P, CT // 2, 2, D], FP8, tag="wk8", name="wk8")
        nc.sync.dma_start(
            out=wk8_sb, in_=wk8[:, :].rearrange("(c i p) n -> p c i n", i=2, p=P)
        )
        qt8_sb = singles.tile([P, CT // 2, 2, S], FP8, tag="qt8", name="qt8")
        qt8_r = qt8[:, :].rearrange("(c i p) n -> p c i n", i=2, p=P)
        nc.sync.dma_start(out=qt8_sb[:, :, :, 0:1024], in_=qt8_r[:, :, :, 0:1024])
        # fp8 ct-pair-interleaved operands for the DoubleRow V projection
        wv8_sb = singles.tile([P, CT // 2, 2, D], FP8, tag="wv8", name="wv8")
        nc.sync.dma_start(
            out=wv8_sb, in_=wv8[:, :].rearrange("(c i p) n -> p c i n", i=2, p=P)
        )
        nc.sync.dma_start(out=qt8_sb[:, :, :, 1024:S], in_=qt8_r[:, :, :, 1024:S])
        wo8_sb = singles.tile([P, CT // 2, 2, D], FP8, tag="wo8", name="wo8")
        nc.sync.dma_start(
            out=wo8_sb, in_=wo8[:, :].rearrange("(c i p) n -> p c i n", i=2, p=P)
        )

        # shift exp by e^-2 so attn weights fit fp8e4m3 (max 448); softmax is
        # shift-invariant -- the ones-column denominator scales identically
        neg2_sb = singles.tile([P, 1], F32, tag="neg2", name="neg2")
        nc.vector.memset(neg2_sb, -2.0)
        ones1 = singles.tile([1, DH], BF16, tag="ones1", name="ones1")
        nc.vector.memset(ones1, 1.0)
        # ones vectors for partition-dim reductions / broadcasts via the PE
        ones_p1 = singles.tile([P, 1], BF16, tag="ones_p1", name="ones_p1")
        nc.vector.memset(ones_p1, 1.0)
        ones_1p = singles.tile([1, P], BF16, tag="ones_1p", name="ones_1p")
        nc.vector.memset(ones_1p, 1.0)
        # rsqrt guard so an all-zero column yields a huge inv_s (saturated
        # q=15 on device, dequantized by s~0 on the host) instead of NaN
        guard = singles.tile([1, 1], F32, tag="guard", name="guard")
        nc.vector.memset(guard, 1e-20)
        # warm the ACT function table while DMAs stream
        warm_t = singles.tile([P, 1], F32, tag="warm", name="warm")
        nc.scalar.activation(warm_t, neg2_sb, AF.Exp)

        q_sb = singles.tile([P, CT, QW], BF16, tag="q_sb", name="q_sb")
        k_sb = singles.tile([P, CT, S], BF16, tag="k_sb", name="k_sb")
        v_sb = singles.tile([P, KT // 2, 2, H, VPAD], FP8, tag="v_sb", name="v_sb")
        av_sb = singles.tile([P, CT // 2, 2, QW], FP8, tag="av_sb", name="av_sb")
        # attn_out accumulator (pre-residual; the host adds Q + b_o exactly)
        x_acc = singles.tile([P, QC, D], F32, tag="x_acc", name="x_acc")
        nc.vector.memset(x_acc, 0.0)

        def q_proj(j):
            psq = ps_pool.tile([P, QW], F32, tag="ps", name="ps")
            for cp in range(CT // 2):
                nc.tensor.matmul(
                    psq,
                    wq8_sb[:, cp, :, j * P : (j + 1) * P],
                    qt8_sb[:, cp, :, 0:QW],
                    start=(cp == 0),
                    stop=(cp == CT // 2 - 1),
                    perf_mode=mybir.MatmulPerfMode.DoubleRow,
                )
            nc.vector.tensor_scalar_add(q_sb[:, j, :], psq, bq_sb[:, j : j + 1])

        def k_proj(j, n4):
            psk = ps_pool.tile([P, 512], F32, tag="ps", name="ps")
            for cp in range(CT // 2):
                nc.tensor.matmul(
                    psk,
                    wk8_sb[:, cp, :, j * P : (j + 1) * P],
                    qt8_sb[:, cp, :, n4 * 512 : (n4 + 1) * 512],
                    start=(cp == 0),
                    stop=(cp == CT // 2 - 1),
                    perf_mode=mybir.MatmulPerfMode.DoubleRow,
                )
            nc.vector.tensor_scalar_add(
                k_sb[:, j, n4 * 512 : (n4 + 1) * 512], psk, bk_sb[:, j : j + 1]
            )

        def v_proj(kt):
            psv = ps_pool.tile([P, D], F32, tag="ps", name="ps")
            for cp in range(CT // 2):
                nc.tensor.matmul(
                    psv[:, 0:512],
                    qt8_sb[:, cp, :, kt * P : (kt + 1) * P],
                    wv8_sb[:, cp, :, 0:512],
                    start=(cp == 0),
                    stop=(cp == CT // 2 - 1),
                    perf_mode=mybir.MatmulPerfMode.DoubleRow,
                )
                nc.tensor.matmul(
                    psv[:, 512:D],
                    qt8_sb[:, cp, :, kt * P : (kt + 1) * P],
                    wv8_sb[:, cp, :, 512:D],
                    start=(cp == 0),
                    stop=(cp == CT // 2 - 1),
                    perf_mode=mybir.MatmulPerfMode.DoubleRow,
                )
            nc.vector.memset(v_sb[:, kt // 2, kt % 2, :, DH : DH + 1], 1.0)
            with nc.allow_low_precision(
                reason="fp8 attn@v operands; error diluted by layernorm"
            ):
                nc.vector.tensor_add(
                    v_sb[:, kt // 2, kt % 2, :, 0:DH],
                    psv.rearrange("p (h d) -> p h d", h=H),
                    bvb.rearrange("p (h d) -> p h d", h=H),
                )

        def o_proj(jp, qc):
            # pair-group jp's (two head pairs) contribution to output rows
            # [qc*128, (qc+1)*128), DoubleRow over the pair interleave,
            # accumulated into x_acc (fp32 SBUF) so PSUM is freed per chunk
            pso = ps_pool.tile([P, D], F32, tag="ps", name="ps")
            nc.tensor.matmul(
                pso[:, 0:512],
                av_sb[:, jp, :, qc * P : (qc + 1) * P],
                wo8_sb[:, jp, :, 0:512],
                start=True,
                stop=True,
                perf_mode=mybir.MatmulPerfMode.DoubleRow,
            )
            nc.tensor.matmul(
                pso[:, 512:D],
                av_sb[:, jp, :, qc * P : (qc + 1) * P],
                wo8_sb[:, jp, :, 512:D],
                start=True,
                stop=True,
                perf_mode=mybir.MatmulPerfMode.DoubleRow,
            )
            nc.vector.tensor_add(x_acc[:, qc, :], x_acc[:, qc, :], pso)

        # initial projections for pair 0 (rest is pipelined into the loop)
        q_proj(0)
        k_proj(0, 0)
        v_proj(0)
        v_proj(1)

        def emit_av(j, ktp, avs, at_tiles):
            # attn@v for k-tile pair ktp, emitted 2 kts after its exps so the
            # in-order PE never blocks waiting on ACT output
            for r in range(2):
                nc.tensor.matmul(
                    avs[r],
                    v_sb[:, ktp, :, 2 * j + r, 0 : DH + 1],
                    at_tiles[ktp][:, :, r * QW : (r + 1) * QW],
                    start=(ktp == 0),
                    stop=(ktp == KT // 2 - 1),
                    perf_mode=mybir.MatmulPerfMode.DoubleRow,
                )

        def emit_norm(j, avs, chunked):
            # normalize: row DH of av is the softmax denominator per q column
            rcs, rbss = [], []
            for r in range(2):
                rc = small_sb.tile([1, QW], BF16, tag="recip", name="recip")
                with nc.allow_low_precision(
                    reason="bf16 softmax denominators; error diluted by layernorm"
                ):
                    nc.vector.reciprocal(rc, avs[r][DH : DH + 1, :])
                rcs.append(rc)
            for r in range(2):
                rbp = ps_pool.tile([DH, QW], F32, tag="ps", name="ps")
                nc.tensor.matmul(rbp, ones1, rcs[r], start=True, stop=True)
                rbs = small_sb.tile([DH, QW], F32, tag="rb", name="rb")
                nc.vector.tensor_copy(rbs, rbp)
                rbss.append(rbs)
            with nc.allow_low_precision(
                reason="fp8 attn output for DoubleRow output projection"
            ):
                if not chunked:
                    for r in range(2):
                        nc.vector.tensor_mul(
                            av_sb[r * DH : (r + 1) * DH, j // 2, j % 2, :],
                            avs[r][0:DH, :],
                            rbss[r],
                        )
                else:
                    for qc in range(QC):
                        for r in range(2):
                            nc.vector.tensor_mul(
                                av_sb[r * DH : (r + 1) * DH, j // 2, j % 2, qc * P : (qc + 1) * P],
                                avs[r][0:DH, qc * P : (qc + 1) * P],
                                rbss[r][:, qc * P : (qc + 1) * P],
                            )

        prev = None  # (j, avs) of the previous pair, normalized inside this one
        for j in range(NPAIR):
            av0 = ps_av.tile([DH + 1, QW], F32, tag="av", name="av")
            av1 = ps_av.tile([DH + 1, QW], F32, tag="av", name="av")
            avs = (av0, av1)
            at_tiles = {}

            for kt in range(KT):
                if j == 0 and kt < KT - 2:
                    v_proj(kt + 2)
                if j == 0 and kt in (1, 3, 5):
                    k_proj(0, (kt + 1) // 2)
                pss = ps_pool.tile([P, 2 * QW], F32, tag="ps", name="ps")
                for r in range(2):
                    nc.tensor.matmul(
                        pss[:, r * QW : (r + 1) * QW],
                        k_sb[r * DH : (r + 1) * DH, j, kt * P : (kt + 1) * P],
                        q_sb[r * DH : (r + 1) * DH, j, :],
                        start=True,
                        stop=True,
                    )
                if kt % 2 == 0:
                    at_tiles[kt // 2] = attn_pool.tile(
                        [P, 2, 2 * QW], FP8, tag="at", name="at"
                    )
                if 1 <= j <= 5 and kt in (3, 6, 10):
                    # offload this tile's exp to DVE via the Schraudolph
                    # bit-trick (uint8 convert saturates negatives to zero)
                    with nc.allow_low_precision(
                        reason="Schraudolph fp8 attn weights; diluted by layernorm"
                    ):
                        nc.vector.tensor_scalar(
                            out=at_tiles[kt // 2][:, kt % 2, :].bitcast(
                                mybir.dt.uint8
                            ),
                            in0=pss,
                            scalar1=SCHRA_A,
                            scalar2=SCHRA_K,
                            op0=mybir.AluOpType.mult,
                            op1=mybir.AluOpType.add,
                        )
                else:
                    nc.scalar.activation(
                        at_tiles[kt // 2][:, kt % 2, :], pss, AF.Exp,
                        scale=SM_SCALE, bias=neg2_sb,
                    )
                if kt == 1 and prev is not None:
                    emit_norm(prev[0], prev[1], chunked=False)
                    prev = None
                if kt % 2 == 1 and kt >= 3:
                    emit_av(j, kt // 2 - 1, avs, at_tiles)
                if j < NPAIR - 1:
                    if kt == 7:
                        q_proj(j + 1)
                    elif kt in (9, 11, 13, 15):
                        k_proj(j + 1, (kt - 9) // 2)
                if j >= 2 and j % 2 == 0 and kt in (4, 7, 12, 14):
                    o_proj(j // 2 - 1, (4, 7, 12, 14).index(kt))

            emit_av(j, KT // 2 - 1, avs, at_tiles)
            prev = (j, avs)

        # last pair: reciprocal + broadcast once, then per-chunk
        # normalize -> output projection -> layernorm, fully pipelined
        lavs = prev[1]
        lrbss = []
        for r in range(2):
            rc = small_sb.tile([1, QW], BF16, tag="recip", name="recip")
            with nc.allow_low_precision(
                reason="bf16 softmax denominators; error diluted by layernorm"
            ):
                nc.vector.reciprocal(rc, lavs[r][DH : DH + 1, :])
            rbp = ps_pool.tile([DH, QW], F32, tag="ps", name="ps")
            nc.tensor.matmul(rbp, ones1, rc, start=True, stop=True)
            rbs = small_sb.tile([DH, QW], F32, tag="rb", name="rb")
            nc.vector.tensor_copy(rbs, rbp)
            lrbss.append(rbs)

        # pass 1 over the chunks: finish attn_out = x_acc + last o_proj and
        # accumulate per-column sums of squares (PE ones-matmul reduces over
        # the partition/row dim; accumulation across chunks lives in SBUF so
        # no PSUM bank is pinned across the loop)
        cs_acc = stats_pool.tile([1, D], F32, tag="cs_acc", name="cs_acc")
        for qc in range(QC):
            with nc.allow_low_precision(
                reason="fp8 attn output for DoubleRow output projection"
            ):
                for r in range(2):
                    nc.vector.tensor_mul(
                        av_sb[r * DH : (r + 1) * DH, NPAIR // 2 - 1, 1, qc * P : (qc + 1) * P],
                        lavs[r][0:DH, qc * P : (qc + 1) * P],
                        lrbss[r][:, qc * P : (qc + 1) * P],
                    )
            pso = ps_pool.tile([P, D], F32, tag="ps", name="ps")
            nc.tensor.matmul(
                pso[:, 0:512],
                av_sb[:, NPAIR // 2 - 1, :, qc * P : (qc + 1) * P],
                wo8_sb[:, NPAIR // 2 - 1, :, 0:512],
                start=True,
                stop=True,
                perf_mode=mybir.MatmulPerfMode.DoubleRow,
            )
            nc.tensor.matmul(
                pso[:, 512:D],
                av_sb[:, NPAIR // 2 - 1, :, qc * P : (qc + 1) * P],
                wo8_sb[:, NPAIR // 2 - 1, :, 512:D],
                start=True,
                stop=True,
                perf_mode=mybir.MatmulPerfMode.DoubleRow,
            )
            x = x_acc[:, qc, :]
            nc.vector.tensor_add(x, x, pso)
            sq = stats_pool.tile([P, D], BF16, tag="sq_scr", name="sq_scr", bufs=2)
            with nc.allow_low_precision(
                reason="bf16 squares only set the int4 quantization scale"
            ):
                nc.scalar.activation(sq, x, AF.Square)
            ps_cs = ps_pool.tile([1, D], F32, tag="ps", name="ps")
            # split at the PSUM bank boundary (512 f32 per bank per matmul)
            nc.tensor.matmul(ps_cs[:, 0:512], ones_p1, sq[:, 0:512], start=True, stop=True)
            nc.tensor.matmul(ps_cs[:, 512:D], ones_p1, sq[:, 512:D], start=True, stop=True)
            if qc == 0:
                nc.vector.tensor_copy(cs_acc, ps_cs)
            else:
                nc.vector.tensor_add(cs_acc, cs_acc, ps_cs)

        # inv_s = (7.5*sqrt(QW)/C4) * rsqrt(colsumsq): sqrt(ssq/A^2) then a
        # reciprocal (bass blocks the Rsqrt ACT function for accuracy); bf16
        # so the host can reproduce the exact divisor from the shipped bits
        srt = stats_pool.tile([1, D], F32, tag="srt", name="srt")
        nc.scalar.activation(srt, cs_acc, AF.Sqrt, scale=RSQ_SCALE, bias=guard)
        inv_s = stats_pool.tile([1, D], BF16, tag="inv_s", name="inv_s")
        with nc.allow_low_precision(
            reason="bf16 quantization scale; host dequantizes with same bits"
        ):
            nc.vector.reciprocal(inv_s, srt)
        ps_b = ps_pool.tile([P, D], F32, tag="ps", name="ps")
        nc.tensor.matmul(ps_b[:, 0:512], ones_1p, inv_s[:, 0:512], start=True, stop=True)
        nc.tensor.matmul(ps_b[:, 512:D], ones_1p, inv_s[:, 512:D], start=True, stop=True)

        # pass 2: quantize to int4 (offset-binary, saturating convert handles
        # clamp-at-0; explicit min handles clamp-at-15), pack column d with
        # column d+384 into one byte, ship
        for qc in range(QC):
            x = x_acc[:, qc, :]
            tt = stats_pool.tile([P, D], F32, tag="tt_scr", name="tt_scr", bufs=2)
            nc.vector.tensor_mul(tt, x, ps_b)
            qu = stats_pool.tile([P, D], mybir.dt.uint8, tag="qu_scr", name="qu_scr", bufs=2)
            with nc.allow_low_precision(
                reason="int4 output quantization, ~0.8% of the 2e-2 gate"
            ):
                nc.vector.tensor_scalar(
                    out=qu, in0=tt, scalar1=7.5, scalar2=15.0,
                    op0=mybir.AluOpType.add, op1=mybir.AluOpType.min,
                )
            qf = stats_pool.tile([P, D], F32, tag="qf_scr", name="qf_scr", bufs=2)
            nc.vector.tensor_copy(qf, qu)  # rounded levels, exact in f32
            pk = stats_pool.tile([P, HD], mybir.dt.uint8, tag="pk_scr", name="pk_scr", bufs=2)
            with nc.allow_low_precision(
                reason="int4 nibble packing; values are exact small integers"
            ):
                nc.vector.scalar_tensor_tensor(
                    out=pk, in0=qf[:, HD:D], scalar=16.0, in1=qf[:, 0:HD],
                    op0=mybir.AluOpType.mult, op1=mybir.AluOpType.add,
                )
            nc.sync.dma_start(out=out[qc * P : (qc + 1) * P, :], in_=pk)
        # ship the bf16 scales as the tail rows, bitcast to uint8 (one DMA
        # per row: the SBUF source lives on a single partition)
        inv_u8 = inv_s.bitcast(mybir.dt.uint8)
        for r in range(OUT_XROWS):
            nc.sync.dma_start(
                out=out[QW + r : QW + r + 1, :],
                in_=inv_u8[:, r * HD : (r + 1) * HD],
            )

    nc.finalize()
    return nc


_CACHE: dict = {}
_BF = ml_dtypes.bfloat16
_FP8 = ml_dtypes.float8_e4m3


def _setup():
    """Build the bass module, the persistent kernel jit and the prep jit."""
    nc = build_nc()
    b2j.install_neuronx_cc_hook()

    partition_name = nc.partition_id_tensor.name if nc.partition_id_tensor else None
    in_names, out_names, out_avals = [], [], []
    for alloc in nc.m.functions[0].allocations:
        if not isinstance(alloc, mybir.MemoryLocationSet):
            continue
        name = alloc.memorylocations[0].name
        if alloc.kind == "ExternalInput":
            if name != partition_name:
                in_names.append(name)
        elif alloc.kind == "ExternalOutput":
            out_names.append(name)
            out_avals.append(
                jax.core.ShapedArray(tuple(alloc.tensor_shape), mybir.dt.np(alloc.dtype))
            )
    n_params = len(in_names)
    n_outs = len(out_names)
    in_names_all = in_names + out_names + ([partition_name] if partition_name else [])
    donate = tuple(range(n_params, n_params + n_outs))

    def _body(*args):
        operands = list(args)
        if partition_name is not None:
            operands.append(b2j.partition_id_tensor())
        outs = b2j._bass_exec_p.bind(
            *operands,
            out_avals=tuple(out_avals),
            in_names=tuple(in_names_all),
            out_names=tuple(out_names),
            lowering_input_output_aliases=(),
            sim_require_finite=True,
            sim_require_nnan=True,
            nc=nc,
        )
        return tuple(outs)

    devices = jax.devices()[:NCORES]
    mesh = Mesh(np.asarray(devices), ("core",))
    pcore = PartitionSpec("core")
    sharding = NamedSharding(mesh, pcore)
    jitted = jax.jit(
        _make_shard_map(
            _body,
            mesh=mesh,
            in_specs=(pcore,) * (n_params + n_outs),
            out_specs=(pcore,) * n_outs,
        ),
        donate_argnums=donate,
        keep_unused=True,
    )

    def _prep(qlocal):
        # per-core [QW, D] fp8 (the core's own query rows) -> rotated fp8
        # Q^T over the full batch sequence + the donated zero output buffer
        g = jax.lax.all_gather(
            qlocal,
            "core",
            axis_index_groups=[[0, 1, 2, 3], [4, 5, 6, 7]],
            tiled=True,
        )  # [S, D] = the core's whole batch, in row order
        q0 = (jax.lax.axis_index("core") % 4) * QW
        g2 = jnp.concatenate([g, g], axis=0)
        rolled = jax.lax.dynamic_slice(g2, (q0, 0), (S, D))
        qt8 = rolled.T
        zeros = jnp.zeros((QW + OUT_XROWS, HD), jnp.uint8)
        return qt8, zeros

    prep = jax.jit(
        _make_shard_map(
            _prep, mesh=mesh, in_specs=(pcore,), out_specs=(pcore,) * 2
        )
    )

    cpu = jax.local_devices(backend="cpu")[0]

    def _cast8(q2d):
        # bf16 -> fp8 two-step, bit-identical to the ml_dtypes casts the
        # weights use, but ~4x faster through XLA's vectorized converts
        return q2d.astype(jnp.bfloat16).astype(jnp.float8_e4m3)

    def _pre(q2d, bo):
        return q2d + bo

    def _post(fetched, qbo, gamma, beta):
        d = fetched.reshape(NCORES, QW + OUT_XROWS, HD)
        packed = d[:, :QW, :]
        inv_s = jax.lax.bitcast_convert_type(
            d[:, QW:, :].reshape(NCORES, D, 2), jnp.bfloat16
        )  # [NCORES, D]
        s = 1.0 / inv_s.astype(jnp.float32)
        lo = (packed & 15).astype(jnp.float32) - 7.5
        hi = (packed >> 4).astype(jnp.float32) - 7.5
        deq = jnp.concatenate([lo, hi], axis=2) * s[:, None, :]
        x = qbo.reshape(NCORES, QW, D) + deq
        x = x.reshape(NCORES * QW, D)
        mu = x.mean(-1, keepdims=True)
        xc = x - mu
        var = (xc * xc).mean(-1, keepdims=True)
        y = xc * jax.lax.rsqrt(var + LN_EPS) * gamma + beta
        return y.reshape(B, S, D)

    with jax.default_device(cpu):
        cast8 = jax.jit(_cast8)
        pre = jax.jit(_pre)
        post = jax.jit(_post)

    _CACHE.update(
        nc=nc,
        jitted=jitted,
        prep=prep,
        cast8=cast8,
        pre=pre,
        post=post,
        cpu=cpu,
        sharding=sharding,
        in_names=in_names,
    )


def _static_inputs(inputs) -> dict:
    """Device-resident replicated weights/biases; re-uploaded if they change."""
    names = ("W_q", "W_k", "W_v", "W_o", "b_q", "b_k", "b_v")
    ids = _CACHE.get("static_ids")
    if ids is not None and all(inputs[k] is ids[k] for k in names):
        return _CACHE["static_dev"]
    host = {k: np.asarray(inputs[k], dtype=np.float32) for k in names}
    cached = _CACHE.get("static_host")
    if cached is not None and all(np.array_equal(host[k], cached[k]) for k in names):
        _CACHE["static_ids"] = {k: inputs[k] for k in names}
        return _CACHE["static_dev"]

    sh = _CACHE["sharding"]
    tiled8 = lambda a: np.broadcast_to(a, (NCORES,) + a.shape).reshape(
        NCORES * a.shape[0], *a.shape[1:]
    )
    wT8 = lambda k: tiled8(
        np.ascontiguousarray(host[k].T).astype(_BF).astype(_FP8)
    )
    dev_host = {
        "wq8": wT8("W_q"),
        "wk8": wT8("W_k"),
        "wv8": wT8("W_v"),
        "wo8": wT8("W_o"),
        "bq": tiled8(host["b_q"]),
        "bk": tiled8(host["b_k"]),
        "bv": tiled8(host["b_v"]),
    }
    keys = list(dev_host)
    devs = jax.device_put([dev_host[k] for k in keys], [sh] * len(keys))
    static_dev = dict(zip(keys, devs))
    _CACHE["static_host"] = host
    _CACHE["static_ids"] = {k: inputs[k] for k in names}
    _CACHE["static_dev"] = static_dev
    return static_dev


def _kernel_traced(inputs) -> np.ndarray:
    """Profiling path through run_bass_kernel_spmd (host-side prep)."""
    Q = np.asarray(inputs["Q"], dtype=np.float32)
    f32 = lambda k: np.ascontiguousarray(np.asarray(inputs[k], dtype=np.float32))
    wT8 = lambda k: np.ascontiguousarray(
        np.asarray(inputs[k], np.float32).T
    ).astype(_BF).astype(_FP8)
    Wq8, Wk8, Wv8, Wo8 = wT8("W_q"), wT8("W_k"), wT8("W_v"), wT8("W_o")
    QT = [np.ascontiguousarray(Q[b].T).astype(_BF).astype(_FP8) for b in range(B)]
    in_maps = []
    for c in range(NCORES):
        b, q0 = c // 4, (c % 4) * QW
        qt_rot = np.ascontiguousarray(
            np.concatenate([QT[b][:, q0:], QT[b][:, :q0]], axis=1)
        )
        in_maps.append(
            {
                "qt8": qt_rot,
                "wq8": Wq8, "wk8": Wk8, "wv8": Wv8, "wo8": Wo8,
                "bq": f32("b_q"), "bk": f32("b_k"), "bv": f32("b_v"),
            }
        )
    res = run_bass_kernel_spmd(
        _CACHE["nc"], in_maps, core_ids=list(range(NCORES)),
        **_CACHE.get("run_kwargs", {}),
    )
    _CACHE["last_result"] = res
    fetched = np.concatenate(
        [np.asarray(res.results[c]["out"]) for c in range(NCORES)], axis=0
    )
    q2d = Q.reshape(NCORES * QW, D)
    with jax.default_device(_CACHE["cpu"]):
        qbo = _CACHE["pre"](q2d, f32("b_o"))
        out = _CACHE["post"](fetched, qbo, f32("ln_gamma"), f32("ln_beta"))
    return np.asarray(out)


def kernel(**inputs) -> np.ndarray:
    if "nc" not in _CACHE:
        _setup()
    if _CACHE.get("run_kwargs"):
        return _kernel_traced(inputs)

    sh = _CACHE["sharding"]

    # issue the Q upload and device-side prep first -- they are the head of
    # the critical path -- then do all remaining host work (static-input
    # resolution, the exact f32 Q + b_o residual) while the device runs.
    # core c <-> global row block c*QW: row order matches Q's (batch-major)
    q2d = np.asarray(inputs["Q"], dtype=np.float32).reshape(NCORES * QW, D)
    with jax.default_device(_CACHE["cpu"]):
        q8 = np.asarray(_CACHE["cast8"](q2d))
    qdev = jax.device_put(q8, sh)
    qt8_d, zeros_d = _CACHE["prep"](qdev)

    static_dev = _static_inputs(inputs)
    feed = dict(static_dev)
    feed["qt8"] = qt8_d
    args = [feed[name] for name in _CACHE["in_names"]]
    (out_d,) = _CACHE["jitted"](*args, zeros_d)
    out_d.copy_to_host_async()  # enqueue D2H before blocking on it

    # overlapped with the device round trip: the exact residual on CPU
    with jax.default_device(_CACHE["cpu"]):
        qbo = _CACHE["pre"](q2d, np.asarray(inputs["b_o"], dtype=np.float32))

    fetched = np.asarray(out_d)  # [NCORES*(QW+4), 384] uint8, blocks
    with jax.default_device(_CACHE["cpu"]):
        out = _CACHE["post"](
            fetched,
            qbo,
            np.asarray(inputs["ln_gamma"], dtype=np.float32),
            np.asarray(inputs["ln_beta"], dtype=np.float32),
        )
    return np.asarray(out)
